# revision 6
# baseline (speedup 1.0000x reference)
"""Cached causal self-attention (single-token decode) on 8 Trainium2 cores.

Sharding: tensor-parallel over heads. Each core owns 4 of the 32 heads:
 - W_qkv rows / b_qkv entries for its heads (q,k,v stacked -> 1536 rows)
 - the KV cache slice for its heads
 - W_out columns for its heads' dims
Each core computes a partial y (16,4096); host sums partials and adds b_out.

On-core dataflow (all fp32):
 - qkv = x @ Wslice.T + bslice on PE (weights pre-transposed on host)
 - per (batch,head) pair the cache is loaded with ONE bulk DMA laid out as
   s = 32p + i (partition p holds 32 consecutive rows = 16KB contiguous in
   DRAM -> near-line-rate descriptors). Rows 0..4063 fill partitions
   0..126; a 1-partition side DMA brings rows 4064..4094 and the new
   token's k/v (from qkv_sb) lands at slot (127, 31) = position 4095, so
   scores/softmax/AV need no tail handling at all.
 - q broadcast via selector-matrix matmul (PSUM); scores on DVE as
   tensor_mul + tensor_reduce over the head dim; exp+per-partition-sum
   fused in one ScalarE activation (scores bounded, no max-subtraction).
 - A@V on PE with V tiles stationary, exp(scores) columns moving; output
   lands as (128 hd, 64 pairs) = the lhsT layout the output projection
   needs. Denominators via ones-matmul column sum.
 - bulk DMAs ride the sync HWDGE ring; small side/token DMAs ride the
   scalar HWDGE ring so they never stall the bulk stream.
"""

import math
from contextlib import ExitStack

import numpy as np

B = 16
H = 32
HD = 128
D = 4096
S_PRIOR = 4095
N_CORES = 8
HC = H // N_CORES          # heads per core
EQ = HC * HD               # 512: per-core q (or k or v) width
E3 = 3 * EQ                # 1536
SCALE = float(1.0 / np.float32(np.sqrt(np.float32(HD))))


def build(b=B, hc=HC, d=D, s_prior=S_PRIOR, reps=1, debug_stage=4):
    import concourse.bass as bass
    import concourse.mybir as mybir
    import concourse.tile as tile
    from concourse import bacc
    from concourse.masks import make_identity

    def bcast_mid(ap2d, n):
        """(P, F) AP -> (P, n, F) AP with a step-0 middle dim (free broadcast)."""
        return bass.AP(
            tensor=ap2d.tensor, offset=ap2d.offset,
            ap=[ap2d.ap[0], [0, n], ap2d.ap[1]],
        )

    f32 = mybir.dt.float32
    eq = hc * HD
    e3 = 3 * eq
    npairs = hc * b
    s_full = s_prior + 1                # 4096 incl. new token
    assert s_full % 128 == 0
    ii = s_full // 128                  # 32 rows per partition
    nbulk = (s_full - ii) // ii         # 127 partitions in the bulk DMA
    nd = d // 128                       # d-tiles for projections
    CH = 512                            # psum free chunk
    nch = (e3 + CH - 1) // CH           # qkv out chunks
    nyj = (d + CH - 1) // CH            # out-proj chunks
    WSUP = 2                            # W_qkv strips per supertile DMA

    nc = bacc.Bacc(trn_type="TRN2")
    xT = nc.dram_tensor("xT", [d, b], f32, kind="ExternalInput")
    wqkvT = nc.dram_tensor("wqkvT", [d, e3], f32, kind="ExternalInput")
    bqkv = nc.dram_tensor("bqkv", [1, e3], f32, kind="ExternalInput")
    kc = nc.dram_tensor("kc", [b, hc, s_prior, HD], f32, kind="ExternalInput")
    vc = nc.dram_tensor("vc", [b, hc, s_prior, HD], f32, kind="ExternalInput")
    woutT = nc.dram_tensor("woutT", [eq, d], f32, kind="ExternalInput")
    y = nc.dram_tensor("y", [b, d], f32, kind="ExternalOutput")

    mult = mybir.AluOpType.mult
    addop = mybir.AluOpType.add

    with tile.TileContext(nc) as tc, ExitStack() as ctx:
        consts = ctx.enter_context(tc.tile_pool(name="consts", bufs=1))
        xsp = ctx.enter_context(tc.tile_pool(name="xsp", bufs=1))
        qkvp = ctx.enter_context(tc.tile_pool(name="qkvp", bufs=1))
        ksp = ctx.enter_context(tc.tile_pool(name="ksp", bufs=2))
        vsp = ctx.enter_context(tc.tile_pool(name="vsp", bufs=2))
        scp = ctx.enter_context(tc.tile_pool(name="scp", bufs=2))
        wp = ctx.enter_context(tc.tile_pool(name="wp", bufs=2))
        junkp = ctx.enter_context(tc.tile_pool(name="junkp", bufs=1))
        statp = ctx.enter_context(tc.tile_pool(name="statp", bufs=1))
        woutp = ctx.enter_context(tc.tile_pool(name="woutp", bufs=hc))
        ychp = ctx.enter_context(tc.tile_pool(name="ychp", bufs=2))
        miscp = ctx.enter_context(tc.tile_pool(name="miscp", bufs=1))
        pavt = ctx.enter_context(tc.tile_pool(name="pavt", bufs=1, space="PSUM"))
        pqb = ctx.enter_context(tc.tile_pool(name="pqb", bufs=1, space="PSUM"))
        pmisc = ctx.enter_context(tc.tile_pool(name="pmisc", bufs=1, space="PSUM"))
        pyp = ctx.enter_context(tc.tile_pool(name="pyp", bufs=2, space="PSUM"))

        def body():
            # ---- constants ----
            ident = consts.tile([128, 128], f32, tag="ident")
            make_identity(nc, ident[:])
            ones_row = consts.tile([1, 128], f32, tag="ones_row")
            nc.vector.memset(ones_row[:], 1.0)
            ones_col = consts.tile([128, 1], f32, tag="ones_col")
            nc.vector.memset(ones_col[:], 1.0)
            # sel_all[k, bb*128 + m] = 1 iff k == bb  (b x b*128)
            sel_all = consts.tile([b, b, 128], f32, tag="sel_all")
            nc.gpsimd.memset(sel_all[:], 0.0)
            nc.gpsimd.affine_select(
                out=sel_all[:],
                in_=sel_all[:],
                compare_op=mybir.AluOpType.not_equal,
                fill=1.0,
                base=0,
                pattern=[[1, b], [0, 128]],
                channel_multiplier=-1,
            )

            # ---- phase 1: qkv = x @ Wslice.T + b ----
            xs = xsp.tile([128, nd, b], f32, tag="xs")
            nc.sync.dma_start(out=xs[:], in_=xT.rearrange("(i p) b -> p i b", p=128))
            bq_sb = consts.tile([1, e3], f32, tag="bq")
            nc.scalar.dma_start(out=bq_sb[:], in_=bqkv[:])
            qkv_sb = qkvp.tile([b, e3], f32, tag="qkv")
            with tc.tile_pool(name="wqp", bufs=2) as wqp, tc.tile_pool(
                name="pqkv", bufs=nch, space="PSUM"
            ) as pqkv:
                psq = [
                    pqkv.tile([b, min(CH, e3 - j * CH)], f32, name="psq", tag="psq")
                    for j in range(nch)
                ]
                for i0 in range(0, nd, WSUP):
                    strip = wqp.tile([128, WSUP, e3], f32, tag="wq_strip")
                    nc.sync.dma_start(
                        out=strip[:],
                        in_=wqkvT[128 * i0 : 128 * (i0 + WSUP), :].rearrange(
                            "(w p) e -> p w e", p=128
                        ),
                    )
                    for w_ in range(WSUP):
                        i = i0 + w_
                        for j in range(nch):
                            w = min(CH, e3 - j * CH)
                            nc.tensor.matmul(
                                psq[j][:],
                                lhsT=xs[:, i, :],
                                rhs=strip[:, w_, j * CH : j * CH + w],
                                start=(i == 0),
                                stop=False,
                            )
                for j in range(nch):
                    w = min(CH, e3 - j * CH)
                    nc.tensor.matmul(
                        psq[j][:],
                        lhsT=ones_row[:, :b],
                        rhs=bq_sb[:, j * CH : j * CH + w],
                        start=False,
                        stop=True,
                    )
                for j in range(nch):
                    w = min(CH, e3 - j * CH)
                    nc.vector.tensor_copy(qkv_sb[:, j * CH : j * CH + w], psq[j][:])

            if debug_stage <= 1:
                ych0 = ychp.tile([b, CH], f32, tag="ych")
                nc.vector.tensor_copy(ych0[:, :CH], qkv_sb[:, :CH])
                nc.sync.dma_start(out=y[:, :CH], in_=ych0[:, :CH])
                return

            # W_out strips: resident; DMAs issued up-front so they slot into
            # the bulk ring behind phase 1's weights, ahead of the KV stream.
            wstrips = []
            for i in range(hc):
                ws = woutp.tile([128, d], f32, tag="wout_strip")
                nc.sync.dma_start(out=ws[:], in_=woutT[128 * i : 128 * (i + 1), :])
                wstrips.append(ws)

            # ---- phase 2: attention over (head, batch) pairs ----
            psum_avT = pavt.tile([128, npairs], f32, tag="avt")
            stats = statp.tile([128, npairs], f32, tag="stats")
            nc.vector.memset(stats[:], 0.0)

            for p in range(npairs if debug_stage >= 3 else 1):
                hh, bb = divmod(p, b)
                qb = pqb.tile([128, 128], f32, tag="qb")
                nc.tensor.matmul(
                    qb[:],
                    lhsT=sel_all[:, bb, :],
                    rhs=qkv_sb[:, hh * HD : (hh + 1) * HD],
                    start=True,
                    stop=True,
                )

                # K cache: bulk (127 partitions x 32 rows) + side (rows
                # 4064..4094) + new token at (127, 31).
                kt = ksp.tile([128, ii, HD], f32, tag="kt")
                nc.sync.dma_start(
                    out=kt[:nbulk, :, :],
                    in_=kc[bb, hh][: nbulk * ii, :].rearrange(
                        "(p i) e -> p i e", p=nbulk
                    ),
                )
                nc.scalar.dma_start(
                    out=kt[nbulk:, : ii - 1, :],
                    in_=kc[bb, hh][nbulk * ii :, :].rearrange(
                        "(p i) e -> p i e", p=1
                    ),
                )
                nc.scalar.dma_start(
                    out=kt[nbulk:, ii - 1, :],
                    in_=qkv_sb[bb : bb + 1, eq + hh * HD : eq + (hh + 1) * HD],
                )

                vt = vsp.tile([128, ii, HD], f32, tag="vt")
                nc.sync.dma_start(
                    out=vt[:nbulk, :, :],
                    in_=vc[bb, hh][: nbulk * ii, :].rearrange(
                        "(p i) e -> p i e", p=nbulk
                    ),
                )
                nc.scalar.dma_start(
                    out=vt[nbulk:, : ii - 1, :],
                    in_=vc[bb, hh][nbulk * ii :, :].rearrange(
                        "(p i) e -> p i e", p=1
                    ),
                )
                nc.scalar.dma_start(
                    out=vt[nbulk:, ii - 1, :],
                    in_=qkv_sb[bb : bb + 1, 2 * eq + hh * HD : 2 * eq + (hh + 1) * HD],
                )

                junk = junkp.tile([128, ii, HD], f32, name="junk", tag="junk")
                nc.vector.tensor_mul(junk[:], kt[:], bcast_mid(qb[:], ii))
                scores = scp.tile([128, ii], f32, tag="scores")
                nc.vector.tensor_reduce(
                    out=scores[:],
                    in_=junk[:],
                    axis=mybir.AxisListType.X,
                    op=addop,
                )

                wt = wp.tile([128, ii], f32, tag="wt")
                nc.scalar.activation(
                    wt[:],
                    scores[:],
                    mybir.ActivationFunctionType.Exp,
                    scale=SCALE,
                    accum_out=stats[:, p : p + 1],
                )

                for i in range(ii):
                    nc.tensor.matmul(
                        psum_avT[:, p : p + 1],
                        lhsT=vt[:, i, :],
                        rhs=wt[:, i : i + 1],
                        start=(i == 0),
                        stop=(i == ii - 1),
                    )

            if debug_stage <= 3:
                ych1 = ychp.tile([b, CH], f32, tag="ych")
                nc.vector.tensor_copy(ych1[:, :npairs], stats[:b, :])
                nc.sync.dma_start(out=y[:, :npairs], in_=ych1[:, :npairs])
                return

            # ---- phase 3: denominators + output projection ----
            denom_ps = pmisc.tile([npairs, 1], f32, tag="pm")
            nc.tensor.matmul(
                denom_ps[:], lhsT=stats[:], rhs=ones_col[:], start=True, stop=True
            )
            denom_sb = miscp.tile([npairs, 1], f32, tag="denom")
            nc.vector.tensor_copy(denom_sb[:], denom_ps[:])
            recip_sb = miscp.tile([npairs, 1], f32, tag="recip")
            nc.vector.reciprocal(recip_sb[:], denom_sb[:])
            recipT_ps = pmisc.tile([1, npairs], f32, tag="pm")
            nc.tensor.transpose(
                recipT_ps[:], recip_sb[:], ident[:npairs, :npairs]
            )
            recipT_sb = miscp.tile([1, npairs], f32, tag="recipT")
            nc.vector.tensor_copy(recipT_sb[:], recipT_ps[:])
            rbc_ps = pmisc.tile([128, npairs], f32, tag="pm")
            nc.tensor.matmul(
                rbc_ps[:], lhsT=ones_row[:], rhs=recipT_sb[:], start=True, stop=True
            )
            rbc_sb = miscp.tile([128, npairs], f32, tag="rbc")
            nc.vector.tensor_copy(rbc_sb[:], rbc_ps[:])
            outT_sb = miscp.tile([128, npairs], f32, tag="outT")
            nc.vector.tensor_mul(outT_sb[:], psum_avT[:], rbc_sb[:])

            for j in range(nyj):
                w = min(CH, d - j * CH)
                psy = pyp.tile([b, CH], f32, tag="py")
                for i in range(hc):
                    nc.tensor.matmul(
                        psy[:, :w],
                        lhsT=outT_sb[:, i * b : (i + 1) * b],
                        rhs=wstrips[i][:, j * CH : j * CH + w],
                        start=(i == 0),
                        stop=(i == hc - 1),
                    )
                ych = ychp.tile([b, CH], f32, tag="ych")
                nc.vector.tensor_copy(ych[:, :w], psy[:, :w])
                nc.sync.dma_start(out=y[:, j * CH : j * CH + w], in_=ych[:, :w])

        if reps == 1:
            body()
        else:
            with tc.For_i(0, reps, 1):
                body()

    nc.compile()
    return nc


def shard_inputs(x_t, k_cache, v_cache, W_qkv, b_qkv, W_out, b_out):
    """Build the 8 per-core input dicts (host-side layout prep)."""
    xTc = np.ascontiguousarray(x_t.reshape(B, D).T)  # (D, B)
    in_maps = []
    for c in range(N_CORES):
        hs = slice(HC * c, HC * (c + 1))
        rq = slice(EQ * c, EQ * (c + 1))
        rk = slice(D + EQ * c, D + EQ * (c + 1))
        rv = slice(2 * D + EQ * c, 2 * D + EQ * (c + 1))
        w_slice = np.concatenate([W_qkv[rq], W_qkv[rk], W_qkv[rv]], axis=0)  # (E3, D)
        b_slice = np.concatenate([b_qkv[rq], b_qkv[rk], b_qkv[rv]])  # (E3,)
        in_maps.append(
            {
                "xT": xTc,
                "wqkvT": np.ascontiguousarray(w_slice.T),  # (D, E3)
                "bqkv": np.ascontiguousarray(b_slice.reshape(1, E3)),
                "kc": np.ascontiguousarray(k_cache[:, hs]),  # (B,HC,S_PRIOR,HD)
                "vc": np.ascontiguousarray(v_cache[:, hs]),
                "woutT": np.ascontiguousarray(W_out[:, EQ * c : EQ * (c + 1)].T),
            }
        )
    return in_maps


_CACHED_NC = None


def kernel(x_t, k_cache, v_cache, W_qkv, b_qkv, W_out, b_out):
    from concourse.bass_utils import run_bass_kernel_spmd

    global _CACHED_NC
    if _CACHED_NC is None:
        _CACHED_NC = build()
    nc = _CACHED_NC

    x_t = np.asarray(x_t, dtype=np.float32)
    k_cache = np.asarray(k_cache, dtype=np.float32)
    v_cache = np.asarray(v_cache, dtype=np.float32)
    W_qkv = np.asarray(W_qkv, dtype=np.float32)
    b_qkv = np.asarray(b_qkv, dtype=np.float32)
    W_out = np.asarray(W_out, dtype=np.float32)
    b_out = np.asarray(b_out, dtype=np.float32)

    in_maps = shard_inputs(x_t, k_cache, v_cache, W_qkv, b_qkv, W_out, b_out)
    res = run_bass_kernel_spmd(nc, in_maps, core_ids=list(range(N_CORES)))
    y = np.zeros((B, D), np.float64)
    for r in res.results:
        y += r["y"].astype(np.float64)
    y = (y + b_out.astype(np.float64)).astype(np.float32)
    return y.reshape(B, 1, D)


# revision 11
# speedup vs baseline: 10.4217x; 10.4217x over previous
"""Cached causal self-attention (single-token decode) on 8 Trainium2 cores.

Sharding: tensor-parallel over heads. Each core owns 4 of the 32 heads:
 - W_qkv rows / b_qkv entries for its heads (q,k,v stacked -> 1536 rows)
 - the KV cache slice for its heads
 - W_out columns for its heads' dims
Each core computes a partial y (16,4096); host sums partials and adds b_out.

On-core dataflow (all fp32):
 - qkv = x @ Wslice.T + bslice on PE (weights pre-transposed on host)
 - per (batch,head) pair the cache is loaded with ONE bulk DMA laid out as
   s = 32p + i (partition p holds 32 consecutive rows = 16KB contiguous in
   DRAM -> near-line-rate descriptors). The host pads each (b,h) cache to
   4096 rows so the bulk DMA covers all 128 partitions — partition counts
   below 128 drop the transfer to a single SDMA engine (~25 GB/s vs
   ~340 GB/s, measured). The new token's k/v (from qkv_sb) then overwrites
   slot (127, 31) = position 4095, so scores/softmax/AV need no tail
   handling at all.
 - q broadcast via selector-matrix matmul (PSUM); scores on DVE as
   tensor_mul + tensor_reduce over the head dim; exp+per-partition-sum
   fused in one ScalarE activation (scores bounded, no max-subtraction).
 - A@V on PE with V tiles stationary, exp(scores) columns moving; output
   lands as (128 hd, 64 pairs) = the lhsT layout the output projection
   needs. Denominators via ones-matmul column sum.
 - bulk DMAs ride the sync HWDGE ring; small side/token DMAs ride the
   scalar HWDGE ring so they never stall the bulk stream.
"""

import math
from contextlib import ExitStack

import numpy as np

B = 16
H = 32
HD = 128
D = 4096
S_PRIOR = 4095
N_CORES = 8
HC = H // N_CORES          # heads per core
EQ = HC * HD               # 512: per-core q (or k or v) width
E3 = 3 * EQ                # 1536
SCALE = float(1.0 / np.float32(np.sqrt(np.float32(HD))))


def build(b=B, hc=HC, d=D, s_prior=S_PRIOR, reps=1, debug_stage=4):
    import concourse.bass as bass
    import concourse.mybir as mybir
    import concourse.tile as tile
    from concourse import bacc
    from concourse.masks import make_identity

    def bcast_mid(ap2d, n):
        """(P, F) AP -> (P, n, F) AP with a step-0 middle dim (free broadcast)."""
        return bass.AP(
            tensor=ap2d.tensor, offset=ap2d.offset,
            ap=[ap2d.ap[0], [0, n], ap2d.ap[1]],
        )

    f32 = mybir.dt.float32
    eq = hc * HD
    e3 = 3 * eq
    npairs = hc * b
    s_full = s_prior + 1                # 4096 incl. new token
    assert s_full % 128 == 0
    ii = s_full // 128                  # 32 rows per partition
    nd = d // 128                       # d-tiles for projections
    CH = 512                            # psum free chunk
    nch = (e3 + CH - 1) // CH           # qkv out chunks
    nyj = (d + CH - 1) // CH            # out-proj chunks
    WSUP = 2                            # W_qkv strips per supertile DMA

    nc = bacc.Bacc(trn_type="TRN2")
    xT = nc.dram_tensor("xT", [d, b], f32, kind="ExternalInput")
    wqkvT = nc.dram_tensor("wqkvT", [d, e3], f32, kind="ExternalInput")
    bqkv = nc.dram_tensor("bqkv", [1, e3], f32, kind="ExternalInput")
    kc = nc.dram_tensor("kc", [b, hc, s_full, HD], f32, kind="ExternalInput")
    vc = nc.dram_tensor("vc", [b, hc, s_full, HD], f32, kind="ExternalInput")
    woutT = nc.dram_tensor("woutT", [eq, d], f32, kind="ExternalInput")
    y = nc.dram_tensor("y", [b, d], f32, kind="ExternalOutput")

    mult = mybir.AluOpType.mult
    addop = mybir.AluOpType.add

    with tile.TileContext(nc) as tc, ExitStack() as ctx:
        consts = ctx.enter_context(tc.tile_pool(name="consts", bufs=1))
        xsp = ctx.enter_context(tc.tile_pool(name="xsp", bufs=1))
        qkvp = ctx.enter_context(tc.tile_pool(name="qkvp", bufs=1))
        ksp = ctx.enter_context(tc.tile_pool(name="ksp", bufs=2))
        vsp = ctx.enter_context(tc.tile_pool(name="vsp", bufs=2))
        scp = ctx.enter_context(tc.tile_pool(name="scp", bufs=2))
        wp = ctx.enter_context(tc.tile_pool(name="wp", bufs=2))
        junkp = ctx.enter_context(tc.tile_pool(name="junkp", bufs=1))
        statp = ctx.enter_context(tc.tile_pool(name="statp", bufs=1))
        woutp = ctx.enter_context(tc.tile_pool(name="woutp", bufs=hc))
        ychp = ctx.enter_context(tc.tile_pool(name="ychp", bufs=2))
        miscp = ctx.enter_context(tc.tile_pool(name="miscp", bufs=1))
        pavt = ctx.enter_context(tc.tile_pool(name="pavt", bufs=1, space="PSUM"))
        pqb = ctx.enter_context(tc.tile_pool(name="pqb", bufs=1, space="PSUM"))
        pmisc = ctx.enter_context(tc.tile_pool(name="pmisc", bufs=1, space="PSUM"))
        pyp = ctx.enter_context(tc.tile_pool(name="pyp", bufs=2, space="PSUM"))

        def body():
            # ---- constants ----
            ident = consts.tile([128, 128], f32, tag="ident")
            make_identity(nc, ident[:])
            ones_row = consts.tile([1, 128], f32, tag="ones_row")
            nc.vector.memset(ones_row[:], 1.0)
            ones_col = consts.tile([128, 1], f32, tag="ones_col")
            nc.vector.memset(ones_col[:], 1.0)
            # sel_all[k, bb*128 + m] = 1 iff k == bb  (b x b*128)
            sel_all = consts.tile([b, b, 128], f32, tag="sel_all")
            nc.gpsimd.memset(sel_all[:], 0.0)
            nc.gpsimd.affine_select(
                out=sel_all[:],
                in_=sel_all[:],
                compare_op=mybir.AluOpType.not_equal,
                fill=1.0,
                base=0,
                pattern=[[1, b], [0, 128]],
                channel_multiplier=-1,
            )

            # ---- phase 1: qkv = x @ Wslice.T + b ----
            xs = xsp.tile([128, nd, b], f32, tag="xs")
            nc.sync.dma_start(out=xs[:], in_=xT.rearrange("(i p) b -> p i b", p=128))
            bq_sb = consts.tile([1, e3], f32, tag="bq")
            nc.scalar.dma_start(out=bq_sb[:], in_=bqkv[:])
            qkv_sb = qkvp.tile([b, e3], f32, tag="qkv")
            with tc.tile_pool(name="wqp", bufs=2) as wqp, tc.tile_pool(
                name="pqkv", bufs=nch, space="PSUM"
            ) as pqkv:
                psq = [
                    pqkv.tile([b, min(CH, e3 - j * CH)], f32, name="psq", tag="psq")
                    for j in range(nch)
                ]
                for i0 in range(0, nd, WSUP):
                    strip = wqp.tile([128, WSUP, e3], f32, tag="wq_strip")
                    nc.sync.dma_start(
                        out=strip[:],
                        in_=wqkvT[128 * i0 : 128 * (i0 + WSUP), :].rearrange(
                            "(w p) e -> p w e", p=128
                        ),
                    )
                    for w_ in range(WSUP):
                        i = i0 + w_
                        for j in range(nch):
                            w = min(CH, e3 - j * CH)
                            nc.tensor.matmul(
                                psq[j][:],
                                lhsT=xs[:, i, :],
                                rhs=strip[:, w_, j * CH : j * CH + w],
                                start=(i == 0),
                                stop=False,
                            )
                for j in range(nch):
                    w = min(CH, e3 - j * CH)
                    nc.tensor.matmul(
                        psq[j][:],
                        lhsT=ones_row[:, :b],
                        rhs=bq_sb[:, j * CH : j * CH + w],
                        start=False,
                        stop=True,
                    )
                for j in range(nch):
                    w = min(CH, e3 - j * CH)
                    nc.vector.tensor_copy(qkv_sb[:, j * CH : j * CH + w], psq[j][:])

            if debug_stage <= 1:
                ych0 = ychp.tile([b, CH], f32, tag="ych")
                nc.vector.tensor_copy(ych0[:, :CH], qkv_sb[:, :CH])
                nc.sync.dma_start(out=y[:, :CH], in_=ych0[:, :CH])
                return

            # W_out strips: resident; DMAs issued up-front so they slot into
            # the bulk ring behind phase 1's weights, ahead of the KV stream.
            wstrips = []
            for i in range(hc):
                ws = woutp.tile([128, d], f32, tag="wout_strip")
                nc.sync.dma_start(out=ws[:], in_=woutT[128 * i : 128 * (i + 1), :])
                wstrips.append(ws)

            # ---- phase 2: attention over (head, batch) pairs ----
            psum_avT = pavt.tile([128, npairs], f32, tag="avt")
            stats = statp.tile([128, npairs], f32, tag="stats")
            nc.vector.memset(stats[:], 0.0)

            for p in range(npairs if debug_stage >= 3 else 1):
                hh, bb = divmod(p, b)
                qb = pqb.tile([128, 128], f32, tag="qb")
                nc.tensor.matmul(
                    qb[:],
                    lhsT=sel_all[:, bb, :],
                    rhs=qkv_sb[:, hh * HD : (hh + 1) * HD],
                    start=True,
                    stop=True,
                )

                # K cache: one 128-partition bulk DMA (host-padded to 4096
                # rows), then the new token overwrites slot (127, 31).
                kt = ksp.tile([128, ii, HD], f32, tag="kt")
                nc.sync.dma_start(
                    out=kt[:],
                    in_=kc[bb, hh].rearrange("(p i) e -> p i e", p=128),
                )
                nc.scalar.dma_start(
                    out=kt[127:, ii - 1, :],
                    in_=qkv_sb[bb : bb + 1, eq + hh * HD : eq + (hh + 1) * HD],
                )

                vt = vsp.tile([128, ii, HD], f32, tag="vt")
                nc.sync.dma_start(
                    out=vt[:],
                    in_=vc[bb, hh].rearrange("(p i) e -> p i e", p=128),
                )
                nc.scalar.dma_start(
                    out=vt[127:, ii - 1, :],
                    in_=qkv_sb[bb : bb + 1, 2 * eq + hh * HD : 2 * eq + (hh + 1) * HD],
                )

                junk = junkp.tile([128, ii, HD], f32, name="junk", tag="junk")
                nc.vector.tensor_mul(junk[:], kt[:], bcast_mid(qb[:], ii))
                scores = scp.tile([128, ii], f32, tag="scores")
                nc.vector.tensor_reduce(
                    out=scores[:],
                    in_=junk[:],
                    axis=mybir.AxisListType.X,
                    op=addop,
                )

                wt = wp.tile([128, ii], f32, tag="wt")
                nc.scalar.activation(
                    wt[:],
                    scores[:],
                    mybir.ActivationFunctionType.Exp,
                    scale=SCALE,
                    accum_out=stats[:, p : p + 1],
                )

                for i in range(ii):
                    nc.tensor.matmul(
                        psum_avT[:, p : p + 1],
                        lhsT=vt[:, i, :],
                        rhs=wt[:, i : i + 1],
                        start=(i == 0),
                        stop=(i == ii - 1),
                    )

            if debug_stage <= 3:
                ych1 = ychp.tile([b, CH], f32, tag="ych")
                nc.vector.tensor_copy(ych1[:, :npairs], stats[:b, :])
                nc.sync.dma_start(out=y[:, :npairs], in_=ych1[:, :npairs])
                return

            # ---- phase 3: denominators + output projection ----
            denom_ps = pmisc.tile([npairs, 1], f32, tag="pm")
            nc.tensor.matmul(
                denom_ps[:], lhsT=stats[:], rhs=ones_col[:], start=True, stop=True
            )
            denom_sb = miscp.tile([npairs, 1], f32, tag="denom")
            nc.vector.tensor_copy(denom_sb[:], denom_ps[:])
            recip_sb = miscp.tile([npairs, 1], f32, tag="recip")
            nc.vector.reciprocal(recip_sb[:], denom_sb[:])
            recipT_ps = pmisc.tile([1, npairs], f32, tag="pm")
            nc.tensor.transpose(
                recipT_ps[:], recip_sb[:], ident[:npairs, :npairs]
            )
            recipT_sb = miscp.tile([1, npairs], f32, tag="recipT")
            nc.vector.tensor_copy(recipT_sb[:], recipT_ps[:])
            rbc_ps = pmisc.tile([128, npairs], f32, tag="pm")
            nc.tensor.matmul(
                rbc_ps[:], lhsT=ones_row[:], rhs=recipT_sb[:], start=True, stop=True
            )
            rbc_sb = miscp.tile([128, npairs], f32, tag="rbc")
            nc.vector.tensor_copy(rbc_sb[:], rbc_ps[:])
            outT_sb = miscp.tile([128, npairs], f32, tag="outT")
            nc.vector.tensor_mul(outT_sb[:], psum_avT[:], rbc_sb[:])

            for j in range(nyj):
                w = min(CH, d - j * CH)
                psy = pyp.tile([b, CH], f32, tag="py")
                for i in range(hc):
                    nc.tensor.matmul(
                        psy[:, :w],
                        lhsT=outT_sb[:, i * b : (i + 1) * b],
                        rhs=wstrips[i][:, j * CH : j * CH + w],
                        start=(i == 0),
                        stop=(i == hc - 1),
                    )
                ych = ychp.tile([b, CH], f32, tag="ych")
                nc.vector.tensor_copy(ych[:, :w], psy[:, :w])
                nc.sync.dma_start(out=y[:, j * CH : j * CH + w], in_=ych[:, :w])

        if reps == 1:
            body()
        else:
            with tc.For_i(0, reps, 1):
                body()

    nc.compile()
    return nc


def shard_inputs(x_t, k_cache, v_cache, W_qkv, b_qkv, W_out, b_out):
    """Build the 8 per-core input dicts (host-side layout prep)."""
    xTc = np.ascontiguousarray(x_t.reshape(B, D).T)  # (D, B)
    in_maps = []
    for c in range(N_CORES):
        hs = slice(HC * c, HC * (c + 1))
        rq = slice(EQ * c, EQ * (c + 1))
        rk = slice(D + EQ * c, D + EQ * (c + 1))
        rv = slice(2 * D + EQ * c, 2 * D + EQ * (c + 1))
        w_slice = np.concatenate([W_qkv[rq], W_qkv[rk], W_qkv[rv]], axis=0)  # (E3, D)
        b_slice = np.concatenate([b_qkv[rq], b_qkv[rk], b_qkv[rv]])  # (E3,)
        # pad each (b,h) cache to 4096 rows: a full-128-partition bulk DMA
        # runs ~13x faster than a 127-partition one; row 4095 is overwritten
        # on-chip by the new token.
        kp = np.zeros((B, HC, S_PRIOR + 1, HD), np.float32)
        kp[:, :, :S_PRIOR] = k_cache[:, hs]
        vp = np.zeros((B, HC, S_PRIOR + 1, HD), np.float32)
        vp[:, :, :S_PRIOR] = v_cache[:, hs]
        in_maps.append(
            {
                "xT": xTc,
                "wqkvT": np.ascontiguousarray(w_slice.T),  # (D, E3)
                "bqkv": np.ascontiguousarray(b_slice.reshape(1, E3)),
                "kc": kp,  # (B,HC,S_PRIOR+1,HD)
                "vc": vp,
                "woutT": np.ascontiguousarray(W_out[:, EQ * c : EQ * (c + 1)].T),
            }
        )
    return in_maps


_CACHED_NC = None


def kernel(x_t, k_cache, v_cache, W_qkv, b_qkv, W_out, b_out):
    from concourse.bass_utils import run_bass_kernel_spmd

    global _CACHED_NC
    if _CACHED_NC is None:
        _CACHED_NC = build()
    nc = _CACHED_NC

    x_t = np.asarray(x_t, dtype=np.float32)
    k_cache = np.asarray(k_cache, dtype=np.float32)
    v_cache = np.asarray(v_cache, dtype=np.float32)
    W_qkv = np.asarray(W_qkv, dtype=np.float32)
    b_qkv = np.asarray(b_qkv, dtype=np.float32)
    W_out = np.asarray(W_out, dtype=np.float32)
    b_out = np.asarray(b_out, dtype=np.float32)

    in_maps = shard_inputs(x_t, k_cache, v_cache, W_qkv, b_qkv, W_out, b_out)
    res = run_bass_kernel_spmd(nc, in_maps, core_ids=list(range(N_CORES)))
    y = np.zeros((B, D), np.float64)
    for r in res.results:
        y += r["y"].astype(np.float64)
    y = (y + b_out.astype(np.float64)).astype(np.float32)
    return y.reshape(B, 1, D)
